# revision 23
# baseline (speedup 1.0000x reference)
"""Trainium2 Bass kernel for nn_EnvironmentSpecificDecoder.

Data-parallel over batch B=32 across 8 NeuronCores (4 batches/core).

v3 design (HAM-aware, PE-column-minimal, all-bf16 matmuls except PSUM):
  stage1 : bf16 pair-packed matmuls, lhsT = zz pair [j, (u,l)], rhs = A
           [j, i] -> z_aggT [(u,l), (pair, i)] in PSUM (bf16 = 1 cyc/row
           even at N=128); vector CAST evacuates to bf16 zzt.
  S23    : fused signal projection + env MLP layer1 (w1s = W_sig@W1[e],
           bf16): 4 matmuls K=64/N=512 into one 4-bank PSUM tile; evac
           hh0 on scalar (ACT Relu+bias), hh1 on vector (TS add+max),
           both -> bf16 h1.
  C1     : corrupt path in bf16; z_corrupt is pre-transposed HOST-side
           into [(u,l), (o,k,i)] so no on-device transposes.  2 MMs/oct
           into a 2-bank PSUM tile, single scalar ACT evac -> bf16 hc.
  S4     : quads are the 4 same-parity t's of an oct, so every rhs is a
           CONTIGUOUS [128,512] slice (strided rhs costs ~150ns/MM).
           2 octs of out2 rows accumulate in one PSUM bank at 32-aligned
           partition slots; one scalar Identity evac per 2 octs + 2
           strided compaction DMAs into dense [64,512] tiles.
  tail   : mu += b2[e]+bo;  sigma = ln(exp(x+b2)+1)+0.01; 2 output DMAs.

The loop is software-pipelined: stage1(g) | S23/C1(g-2) | S4(g-3), so
every matmul's input was evacuated >=1 full oct-block earlier and the
PE never waits on the (queue-latency-laden) scalar/vector engines.
Sustained PE activity keeps the HAM clock gate at 8/8 (2.4 GHz).
"""
import numpy as np
import ml_dtypes

N_CORES = 8
NB = 4          # batches per core
T = 64
D = 128
L = 64
H = 256
H2 = 128
NE = 8
NOCT = T // 8   # octs per batch (8 t's per oct)

_CACHE = {}


def _bf16(x: np.ndarray) -> np.ndarray:
    return np.ascontiguousarray(np.asarray(x, dtype=np.float32)).astype(
        ml_dtypes.bfloat16)


def _build():
    import concourse.bacc as bacc
    import concourse.bass as bass
    import concourse.mybir as mybir
    from concourse.tile import TileContext

    F32 = mybir.dt.float32
    BF16 = mybir.dt.bfloat16
    AF = mybir.ActivationFunctionType
    ADD = mybir.AluOpType.add
    MAX = mybir.AluOpType.max

    nc = bacc.Bacc("TRN2", target_bir_lowering=False, debug=False)

    # signal, pair-packed: [b, j, t*L] (col = t*64+l)
    zzi_d = nc.dram_tensor("zzi", [NB, D, T * L], BF16, kind="ExternalInput")
    # corrupt, pre-transposed: [b, (u,l), (o,k,i)]
    zci_d = nc.dram_tensor("zci", [NB, D, NOCT * 512], BF16,
                           kind="ExternalInput")
    ai_d = nc.dram_tensor("ai", [D, D], BF16, kind="ExternalInput")
    reg_d = nc.dram_tensor("reg", [1, NB], mybir.dt.int32,
                           kind="ExternalInput")
    w1s_d = nc.dram_tensor("w1s", [NE, D, H], BF16, kind="ExternalInput")
    b1s_d = nc.dram_tensor("b1s", [NE, D, 2], F32, kind="ExternalInput")
    w2p_d = nc.dram_tensor("w2p", [NE, D, 2, 2], BF16, kind="ExternalInput")
    # out2 bias at S4 psum slot partitions: [e, 32*s + k] = b2[e,k] (+bo)
    bq_d = nc.dram_tensor("bq", [NE, D, 1], F32, kind="ExternalInput")
    wc_d = nc.dram_tensor("wc", [D, H2], BF16, kind="ExternalInput")
    bc_d = nc.dram_tensor("bc", [H2, 1], F32, kind="ExternalInput")
    wo_d = nc.dram_tensor("wo", [H2, 1], BF16, kind="ExternalInput")

    mu_d = nc.dram_tensor("mu", [NB, T, D], F32, kind="ExternalOutput")
    sg_d = nc.dram_tensor("sg", [NB, T, D], F32, kind="ExternalOutput")

    NG = NB * NOCT  # 32 global octs

    with TileContext(nc) as tc:
        with (
            tc.tile_pool(name="const", bufs=1) as constp,
            tc.tile_pool(name="zz", bufs=2) as zzp,
            tc.tile_pool(name="zc", bufs=2) as zcp,
            tc.tile_pool(name="zzt", bufs=3) as zztp,
            tc.tile_pool(name="h1", bufs=3) as h1p,
            tc.tile_pool(name="hc", bufs=3) as hcp,
            tc.tile_pool(name="sto", bufs=2) as stop,
            tc.tile_pool(name="fin", bufs=1) as finp,
            tc.tile_pool(name="ps1", bufs=1, space="PSUM") as ps1,
            tc.tile_pool(name="ps23", bufs=1, space="PSUM") as ps23,
            tc.tile_pool(name="psc", bufs=1, space="PSUM") as psc,
            tc.tile_pool(name="ps4", bufs=1, space="PSUM") as ps4,
        ):
            # ---- inputs needed by the PE first ----
            reg_sb = constp.tile([1, NB], mybir.dt.int32)
            nc.sync.dma_start(reg_sb[:], reg_d[:])
            ai_sb = constp.tile([D, D], BF16)
            nc.sync.dma_start(ai_sb[:], ai_d[:])

            zz_sb = {}
            zc_sb = {}

            def load_b(b):
                # split so the first octs' data lands (and compute starts)
                # before the whole 1 MB transfer completes
                zz = zzp.tile([D, T * L], BF16, tag="zz")
                nc.sync.dma_start(zz[:, 0:1024], zzi_d[b][:, 0:1024])
                nc.sync.dma_start(zz[:, 1024:4096], zzi_d[b][:, 1024:4096])
                zc = zcp.tile([D, NOCT * 512], BF16, tag="zc")
                nc.sync.dma_start(zc[:], zci_d[b])
                zz_sb[b] = zz
                zc_sb[b] = zc

            load_b(0)

            # ---- static + dispatched weights (regime -> env) ----
            wc_sb = constp.tile([D, H2], BF16)       # Wc stacked twice
            nc.sync.dma_start(wc_sb[:], wc_d[:])
            wo_sb = constp.tile([H2, 1], BF16)
            nc.sync.dma_start(wo_sb[:], wo_d[:])
            bc_sb = constp.tile([H2, 1], F32)
            nc.sync.dma_start(bc_sb[:], bc_d[:])

            w1s_sb, b1s_sb, w2_sb, bq_sb = [], [], [], []
            for b in range(NB):
                e = nc.values_load(
                    reg_sb[0:1, b : b + 1],
                    engines=[mybir.EngineType.SP],
                    min_val=0, max_val=NE - 1,
                    skip_runtime_bounds_check=True,
                )
                w1 = constp.tile([D, H], BF16, name=f"w1s{b}", tag=f"w1s{b}")
                nc.sync.dma_start(
                    w1[:], w1s_d[bass.ds(e, 1)].rearrange("o p h -> (o p) h")
                )
                b1 = constp.tile([D, 2], F32, name=f"b1s{b}", tag=f"b1s{b}")
                nc.sync.dma_start(
                    b1[:], b1s_d[bass.ds(e, 1)].rearrange("o p h -> (o p) h")
                )
                w2 = constp.tile([D, 2, 2], BF16, name=f"w2{b}", tag=f"w2{b}")
                nc.sync.dma_start(
                    w2[:], w2p_d[bass.ds(e, 1)].rearrange("o p a k -> (o p) a k")
                )
                bq = constp.tile([D, 1], F32, name=f"bq{b}", tag=f"bq{b}")
                nc.sync.dma_start(
                    bq[:], bq_d[bass.ds(e, 1)].rearrange("o p k -> (o p) k")
                )
                w1s_sb.append(w1)
                b1s_sb.append(b1)
                w2_sb.append(w2)
                bq_sb.append(bq)

            st_mu = finp.tile([NB * 16, 512], F32)
            st_sig = finp.tile([NB * 16, 512], F32)

            # ---- PE warmup: keep the HAM activity monitor busy while the
            # first zz transfer is in flight, so real matmuls start at the
            # warm 2.4 GHz clock instead of the cold 1.2 GHz one.  Uses the
            # S4 PSUM bank, which carries no live data until block 3.
            warm = ps4.tile([D, 512], F32, name="warm", tag="p4")

            def fillers(n):
                for k in range(n):
                    nc.tensor.matmul(
                        warm[:, 128 * (k % 4) : 128 * (k % 4 + 1)],
                        ai_sb[:], ai_sb[:], start=True, stop=True,
                    )

            fillers(30)

            zzt_g = {}
            h1_g = {}
            hc_g = {}
            p4_cur = [None]

            def stage1(g):
                b, o = divmod(g, NOCT)
                zz = zz_sb[b]
                p1 = ps1.tile([D, 512], F32, tag="p1")
                for k in range(4):
                    pr = o * 4 + k
                    nc.tensor.matmul(
                        p1[:, 128 * k : 128 * (k + 1)],
                        zz[:, 128 * pr : 128 * (pr + 1)],
                        ai_sb[:],
                        start=True, stop=True,
                    )
                zzt = zztp.tile([D, 512], BF16, tag="zzt")
                nc.vector.tensor_copy(zzt[:], p1[:])
                zzt_g[g] = zzt

            def s23_c1(g):
                b, o = divmod(g, NOCT)
                zzt = zzt_g.pop(g)
                h1 = h1p.tile([D, 2048], BF16, tag="h1")
                p23 = ps23.tile([D, 2048], F32, tag="p23")
                for hh in range(2):
                    for par in range(2):
                        nc.tensor.matmul(
                            p23[:, 1024 * hh + 512 * par :
                                1024 * hh + 512 * (par + 1)],
                            w1s_sb[b][64 * par : 64 * par + 64,
                                      128 * hh : 128 * (hh + 1)],
                            zzt[64 * par : 64 * par + 64, :],
                            start=True, stop=True,
                        )
                nc.scalar.activation(
                    h1[:, 0:1024], p23[:, 0:1024], AF.Relu,
                    bias=b1s_sb[b][:, 0:1],
                )
                nc.vector.tensor_scalar(
                    h1[:, 1024:2048], p23[:, 1024:2048],
                    b1s_sb[b][:, 1:2], 0.0, ADD, MAX,
                )
                h1_g[g] = h1

                zc = zc_sb[b]
                hc = hcp.tile([D, 1024], BF16, tag="hc")
                pc = psc.tile([D, 1024], F32, tag="pc")
                for par in range(2):
                    nc.tensor.matmul(
                        pc[:, 512 * par : 512 * (par + 1)],
                        wc_sb[64 * par : 64 * par + 64, :],
                        zc[64 * par : 64 * par + 64, 512 * o : 512 * (o + 1)],
                        start=True, stop=True,
                    )
                nc.scalar.activation(hc[:], pc[:], AF.Relu, bias=bc_sb[:, 0:1])
                hc_g[g] = hc

            def s4(g):
                b, o = divmod(g, NOCT)
                h1 = h1_g.pop(g)
                hc = hc_g.pop(g)
                if g % 2 == 0:
                    p4_cur[0] = ps4.tile([D, 512], F32, name="p4",
                                         tag="p4")
                p4 = p4_cur[0]
                # the two accumulation chains (par 0/1) target different PE
                # column strips; interleaving lets them run concurrently
                def mm(par, step, start, stop):
                    s = (g % 2) * 2 + par
                    tp = (0, 32 * s)
                    if step == 1:
                        nc.tensor.matmul(
                            p4[32 * s : 32 * s + 1, :], wo_sb[:],
                            hc[:, 512 * par : 512 * (par + 1)],
                            start=start, stop=stop, tile_position=tp,
                        )
                    else:
                        hh = step // 2
                        nc.tensor.matmul(
                            p4[32 * s : 32 * s + 2, :], w2_sb[b][:, hh, :],
                            h1[:, 1024 * hh + 512 * par :
                               1024 * hh + 512 * (par + 1)],
                            start=start, stop=stop, tile_position=tp,
                        )
                for par in range(2):
                    for step in range(3):
                        mm(par, step, step == 0, step == 2)
                if g % 2 == 1:
                    sto = stop.tile([D, 512], F32, tag="sto")
                    nc.vector.tensor_scalar(
                        sto[:], p4[:], bq_sb[b][:, 0:1], 0.0, ADD,
                        mybir.AluOpType.bypass,
                    )
                    stov = sto[:].rearrange("(s r) f -> s r f", r=32)
                    r0 = b * 16 + (o - 1) * 2
                    nc.sync.dma_start(st_mu[r0 : r0 + 4, :], stov[:, 0])
                    nc.sync.dma_start(st_sig[r0 : r0 + 4, :], stov[:, 1])

            # filler MMs bridge the PE-idle stretches while the software
            # pipeline fills, so the HAM never re-throttles
            prolog_fill = {0: 16, 1: 12, 2: 8, 3: 2}
            for g in range(NG + 3):
                if g < NG:
                    b, o = divmod(g, NOCT)
                    if o == 4 and b + 1 < NB:
                        load_b(b + 1)
                    stage1(g)
                if g in prolog_fill:
                    fillers(prolog_fill[g])
                if 2 <= g < NG + 2:
                    s23_c1(g - 2)
                if 3 <= g:
                    s4(g - 3)

            # ---- tail: softplus + outputs (b2/bo already added at evac) ----
            ex = finp.tile([NB * 16, 512], F32)
            nc.scalar.activation(ex[:], st_sig[:], AF.Exp)
            nc.scalar.activation(st_sig[:], ex[:], AF.Ln, bias=1.0)
            nc.vector.tensor_scalar_add(st_sig[:], st_sig[:], 0.01)

            # st row r = b*16 + o*2 + u covers t = o*8 + u*4 + j, cols (j, i)
            nc.sync.dma_start(
                mu_d[:].rearrange("b (qb tq) i -> (b qb) tq i", tq=4),
                st_mu[:].rearrange("p (tq i) -> p tq i", i=D),
            )
            nc.sync.dma_start(
                sg_d[:].rearrange("b (qb tq) i -> (b qb) tq i", tq=4),
                st_sig[:].rearrange("p (tq i) -> p tq i", i=D),
            )

    nc.compile()
    return nc


def _get_nc():
    if "nc" not in _CACHE:
        _CACHE["nc"] = _build()
    return _CACHE["nc"]


def _prepare_in_maps(z_signal, z_corrupt, A, regime, W_sig, b_sig, W1e, b1e,
                     W2e, b2e, Wc, bc, Wo, bo):
    z_signal = np.asarray(z_signal, dtype=np.float32)
    z_corrupt = np.asarray(z_corrupt, dtype=np.float32)
    A = np.asarray(A, dtype=np.float32)
    regime = np.asarray(regime)
    W_sig = np.asarray(W_sig, dtype=np.float32)
    b_sig = np.asarray(b_sig, dtype=np.float32)
    W1e = np.asarray(W1e, dtype=np.float32)
    b1e = np.asarray(b1e, dtype=np.float32)
    W2e = np.asarray(W2e, dtype=np.float32)
    b2e = np.asarray(b2e, dtype=np.float32)
    Wc = np.asarray(Wc, dtype=np.float32)
    bc = np.asarray(bc, dtype=np.float32)
    Wo = np.asarray(Wo, dtype=np.float32)
    bo = np.asarray(bo, dtype=np.float32)

    eidx = np.where(regime >= NE, 0, regime).astype(np.int32)

    # ---- host weight transforms (env tables, replicated to all cores) ----
    ai = _bf16(A)                                                  # [D, D]
    w1s_half = _bf16(np.einsum("lh,ehk->elk", W_sig, W1e))         # [E, L, H]
    w1s = np.concatenate([w1s_half, w1s_half], axis=1)             # [E, D, H]
    b1s_full = np.einsum("h,ehk->ek", b_sig, W1e) + b1e            # [E, H]
    b1s = np.ascontiguousarray(
        b1s_full.reshape(NE, 2, D).transpose(0, 2, 1))             # [E, D, 2]
    w2p = _bf16(np.ascontiguousarray(
        W2e.reshape(NE, 2, D, 2).transpose(0, 2, 1, 3)))           # [E,D,2,2]
    bq = np.zeros((NE, D, 1), dtype=np.float32)
    for s in range(4):
        bq[:, 32 * s, 0] = b2e[:, 0] + bo[0]
        bq[:, 32 * s + 1, 0] = b2e[:, 1]
    wc_r = _bf16(np.concatenate([Wc] * 2, axis=0))                 # [D, H2]
    wo_r = _bf16(Wo)                                               # [H2, 1]
    bc_r = np.ascontiguousarray(bc[:, None])                       # [H2, 1]

    in_maps = []
    for c in range(N_CORES):
        b0 = c * NB
        zs = z_signal[b0 : b0 + NB]
        zc = z_corrupt[b0 : b0 + NB]
        # signal pairs are (t, t+4) within an oct: pair pr = o*4+j holds
        # t = o*8 + u*4 + j at col pr*128 + u*64 + l
        zzt_ = zs.transpose(0, 2, 1, 3)                  # b,i,t,l
        zzt_ = zzt_.reshape(NB, D, NOCT, 2, 4, L)        # b,i,o,u,j,l
        zzi = _bf16(np.ascontiguousarray(
            zzt_.transpose(0, 1, 2, 4, 3, 5)).reshape(NB, D, T * L))
        # corrupt: [nb, T, D, L] -> [nb, (u,l), (o,j,i)], t = o*8 + u*4 + j
        zct = zc.reshape(NB, NOCT, 2, 4, D, L)           # b,o,u,j,i,l
        zct = zct.transpose(0, 2, 5, 1, 3, 4)            # b,u,l,o,j,i
        zci = _bf16(np.ascontiguousarray(zct).reshape(NB, D, NOCT * 512))
        in_maps.append({
            "zzi": zzi,
            "zci": zci,
            "ai": ai,
            "reg": eidx[None, b0 : b0 + NB],
            "w1s": w1s,
            "b1s": b1s,
            "w2p": w2p,
            "bq": bq,
            "wc": wc_r,
            "bc": bc_r,
            "wo": wo_r,
        })
    return in_maps


def kernel(z_signal, z_corrupt, A, regime, W_sig, b_sig, W1e, b1e, W2e, b2e,
           Wc, bc, Wo, bo):
    from concourse.bass_utils import run_bass_kernel_spmd

    in_maps = _prepare_in_maps(z_signal, z_corrupt, A, regime, W_sig, b_sig,
                               W1e, b1e, W2e, b2e, Wc, bc, Wo, bo)
    nc = _get_nc()
    res = run_bass_kernel_spmd(nc, in_maps, core_ids=list(range(N_CORES)))

    mu = np.concatenate([r["mu"] for r in res.results], axis=0)
    sigma = np.concatenate([r["sg"] for r in res.results], axis=0)
    return mu, sigma


def run_traced(inputs_np):
    from concourse.bass_utils import run_bass_kernel_spmd

    in_maps = _prepare_in_maps(**inputs_np)
    nc = _get_nc()
    return run_bass_kernel_spmd(
        nc, in_maps, core_ids=list(range(N_CORES)), trace=True
    )


# revision 26
# speedup vs baseline: 1.0166x; 1.0166x over previous
"""Trainium2 Bass kernel for nn_EnvironmentSpecificDecoder.

Data-parallel over batch B=32 across 8 NeuronCores (4 batches/core).

v3 design (HAM-aware, PE-column-minimal, all-bf16 matmuls except PSUM):
  stage1 : bf16 pair-packed matmuls, lhsT = zz pair [j, (u,l)], rhs = A
           [j, i] -> z_aggT [(u,l), (pair, i)] in PSUM (bf16 = 1 cyc/row
           even at N=128); vector CAST evacuates to bf16 zzt.
  S23    : fused signal projection + env MLP layer1 (w1s = W_sig@W1[e],
           bf16): 4 matmuls K=64/N=512 into one 4-bank PSUM tile; evac
           hh0 on scalar (ACT Relu+bias), hh1 on vector (TS add+max),
           both -> bf16 h1.
  C1     : corrupt path in bf16; z_corrupt is pre-transposed HOST-side
           into [(u,l), (o,k,i)] so no on-device transposes.  2 MMs/oct
           into a 2-bank PSUM tile, single scalar ACT evac -> bf16 hc.
  S4     : quads are the 4 same-parity t's of an oct, so every rhs is a
           CONTIGUOUS [128,512] slice (strided rhs costs ~150ns/MM).
           2 octs of out2 rows accumulate in one PSUM bank at 32-aligned
           partition slots; one scalar Identity evac per 2 octs + 2
           strided compaction DMAs into dense [64,512] tiles.
  tail   : mu += b2[e]+bo;  sigma = ln(exp(x+b2)+1)+0.01; 2 output DMAs.

The loop is software-pipelined: stage1(g) | S23/C1(g-2) | S4(g-3), so
every matmul's input was evacuated >=1 full oct-block earlier and the
PE never waits on the (queue-latency-laden) scalar/vector engines.
Sustained PE activity keeps the HAM clock gate at 8/8 (2.4 GHz).
"""
import numpy as np
import ml_dtypes

N_CORES = 8
NB = 4          # batches per core
T = 64
D = 128
L = 64
H = 256
H2 = 128
NE = 8
NOCT = T // 8   # octs per batch (8 t's per oct)

_CACHE = {}


def _bf16(x: np.ndarray) -> np.ndarray:
    return np.ascontiguousarray(np.asarray(x, dtype=np.float32)).astype(
        ml_dtypes.bfloat16)


def _build():
    import concourse.bacc as bacc
    import concourse.bass as bass
    import concourse.mybir as mybir
    from concourse.tile import TileContext

    F32 = mybir.dt.float32
    BF16 = mybir.dt.bfloat16
    AF = mybir.ActivationFunctionType
    ADD = mybir.AluOpType.add
    MAX = mybir.AluOpType.max

    nc = bacc.Bacc("TRN2", target_bir_lowering=False, debug=False)

    # signal, pair-packed: [b, j, t*L] (col = t*64+l)
    zzi_d = nc.dram_tensor("zzi", [NB, D, T * L], BF16, kind="ExternalInput")
    # corrupt, pre-transposed: [b, (u,l), (o,k,i)]
    zci_d = nc.dram_tensor("zci", [NB, D, NOCT * 512], BF16,
                           kind="ExternalInput")
    ai_d = nc.dram_tensor("ai", [D, D], BF16, kind="ExternalInput")
    reg_d = nc.dram_tensor("reg", [1, NB], mybir.dt.int32,
                           kind="ExternalInput")
    w1s_d = nc.dram_tensor("w1s", [NE, D, H], BF16, kind="ExternalInput")
    b1s_d = nc.dram_tensor("b1s", [NE, D, 2], F32, kind="ExternalInput")
    w2p_d = nc.dram_tensor("w2p", [NE, D, 2, 2], BF16, kind="ExternalInput")
    # out2 bias at S4 psum slot partitions: [e, 32*s + k] = b2[e,k] (+bo)
    bq_d = nc.dram_tensor("bq", [NE, D, 1], F32, kind="ExternalInput")
    wc_d = nc.dram_tensor("wc", [D, H2], BF16, kind="ExternalInput")
    bc_d = nc.dram_tensor("bc", [H2, 1], F32, kind="ExternalInput")
    wo_d = nc.dram_tensor("wo", [H2, 1], BF16, kind="ExternalInput")

    mu_d = nc.dram_tensor("mu", [NB, T, D], F32, kind="ExternalOutput")
    sg_d = nc.dram_tensor("sg", [NB, T, D], F32, kind="ExternalOutput")

    NG = NB * NOCT  # 32 global octs

    with TileContext(nc) as tc:
        with (
            tc.tile_pool(name="const", bufs=1) as constp,
            tc.tile_pool(name="zz", bufs=2) as zzp,
            tc.tile_pool(name="zc", bufs=2) as zcp,
            tc.tile_pool(name="zzt", bufs=3) as zztp,
            tc.tile_pool(name="h1", bufs=3) as h1p,
            tc.tile_pool(name="hc", bufs=3) as hcp,
            tc.tile_pool(name="sto", bufs=2) as stop,
            tc.tile_pool(name="fin", bufs=1) as finp,
            tc.tile_pool(name="ps1", bufs=1, space="PSUM") as ps1,
            tc.tile_pool(name="ps23", bufs=1, space="PSUM") as ps23,
            tc.tile_pool(name="psc", bufs=1, space="PSUM") as psc,
            tc.tile_pool(name="ps4", bufs=1, space="PSUM") as ps4,
        ):
            # ---- inputs needed by the PE first ----
            reg_sb = constp.tile([1, NB], mybir.dt.int32)
            nc.sync.dma_start(reg_sb[:], reg_d[:])
            ai_sb = constp.tile([D, D], BF16)
            nc.sync.dma_start(ai_sb[:], ai_d[:])

            zz_sb = {}
            zc_sb = {}

            def load_b(b):
                # split so the first octs' data lands (and compute starts)
                # before the whole 1 MB transfer completes
                zz = zzp.tile([D, T * L], BF16, tag="zz")
                nc.sync.dma_start(zz[:, 0:1024], zzi_d[b][:, 0:1024])
                nc.sync.dma_start(zz[:, 1024:4096], zzi_d[b][:, 1024:4096])
                zc = zcp.tile([D, NOCT * 512], BF16, tag="zc")
                nc.sync.dma_start(zc[:], zci_d[b])
                zz_sb[b] = zz
                zc_sb[b] = zc

            zz0 = zzp.tile([D, T * L], BF16, tag="zz")
            nc.sync.dma_start(zz0[:, 0:1024], zzi_d[0][:, 0:1024])
            zz_sb[0] = zz0

            # batch-0 dispatched weights right away: S23(0)/C1(0) run only
            # ~2 blocks after stage1(0) and must not wait on these
            w1s_sb, b1s_sb, w2_sb, bq_sb, e_v = [], [], [], [], []
            def dispatch_b(b):
                e = nc.values_load(
                    reg_sb[0:1, b : b + 1],
                    engines=[mybir.EngineType.SP],
                    min_val=0, max_val=NE - 1,
                    skip_runtime_bounds_check=True,
                )
                w1 = constp.tile([D, H], BF16, name=f"w1s{b}", tag=f"w1s{b}")
                nc.sync.dma_start(
                    w1[:], w1s_d[bass.ds(e, 1)].rearrange("o p h -> (o p) h")
                )
                b1 = constp.tile([D, 2], F32, name=f"b1s{b}", tag=f"b1s{b}")
                nc.sync.dma_start(
                    b1[:], b1s_d[bass.ds(e, 1)].rearrange("o p h -> (o p) h")
                )
                w1s_sb.append(w1)
                b1s_sb.append(b1)
                e_v.append(e)

            dispatch_b(0)
            wc_sb = constp.tile([D, H2], BF16)       # Wc stacked twice
            nc.sync.dma_start(wc_sb[:], wc_d[:])
            bc_sb = constp.tile([H2, 1], F32)
            nc.sync.dma_start(bc_sb[:], bc_d[:])

            nc.sync.dma_start(zz0[:, 1024:4096], zzi_d[0][:, 1024:4096])
            zc0 = zcp.tile([D, NOCT * 512], BF16, tag="zc")
            nc.sync.dma_start(zc0[:], zci_d[0])
            zc_sb[0] = zc0

            wo_sb = constp.tile([H2, 1], BF16)
            nc.sync.dma_start(wo_sb[:], wo_d[:])
            for b in range(1, NB):
                dispatch_b(b)
            for b in range(NB):
                e = e_v[b]
                w2 = constp.tile([D, 2, 2], BF16, name=f"w2{b}", tag=f"w2{b}")
                nc.sync.dma_start(
                    w2[:], w2p_d[bass.ds(e, 1)].rearrange("o p a k -> (o p) a k")
                )
                bq = constp.tile([D, 1], F32, name=f"bq{b}", tag=f"bq{b}")
                nc.sync.dma_start(
                    bq[:], bq_d[bass.ds(e, 1)].rearrange("o p k -> (o p) k")
                )
                w2_sb.append(w2)
                bq_sb.append(bq)

            st_mu = finp.tile([NB * 16, 512], F32)
            st_sig = finp.tile([NB * 16, 512], F32)

            # ---- PE warmup: keep the HAM activity monitor busy while the
            # first zz transfer is in flight, so real matmuls start at the
            # warm 2.4 GHz clock instead of the cold 1.2 GHz one.  Uses the
            # S4 PSUM bank, which carries no live data until block 3.
            warm = ps4.tile([D, 512], F32, name="warm", tag="p4")

            def fillers(n):
                for k in range(n):
                    nc.tensor.matmul(
                        warm[:, 128 * (k % 4) : 128 * (k % 4 + 1)],
                        ai_sb[:], ai_sb[:], start=True, stop=True,
                    )

            fillers(30)

            zzt_g = {}
            h1_g = {}
            hc_g = {}
            p4_cur = [None]

            def stage1(g):
                b, o = divmod(g, NOCT)
                zz = zz_sb[b]
                p1 = ps1.tile([D, 512], F32, tag="p1")
                for k in range(4):
                    pr = o * 4 + k
                    nc.tensor.matmul(
                        p1[:, 128 * k : 128 * (k + 1)],
                        zz[:, 128 * pr : 128 * (pr + 1)],
                        ai_sb[:],
                        start=True, stop=True,
                    )
                zzt = zztp.tile([D, 512], BF16, tag="zzt")
                nc.vector.tensor_copy(zzt[:], p1[:])
                zzt_g[g] = zzt

            def s23_c1(g):
                b, o = divmod(g, NOCT)
                zzt = zzt_g.pop(g)
                h1 = h1p.tile([D, 2048], BF16, tag="h1")
                p23 = ps23.tile([D, 2048], F32, tag="p23")
                for hh in range(2):
                    for par in range(2):
                        nc.tensor.matmul(
                            p23[:, 1024 * hh + 512 * par :
                                1024 * hh + 512 * (par + 1)],
                            w1s_sb[b][64 * par : 64 * par + 64,
                                      128 * hh : 128 * (hh + 1)],
                            zzt[64 * par : 64 * par + 64, :],
                            start=True, stop=True,
                        )
                nc.scalar.activation(
                    h1[:, 0:1024], p23[:, 0:1024], AF.Relu,
                    bias=b1s_sb[b][:, 0:1],
                )
                nc.vector.tensor_scalar(
                    h1[:, 1024:2048], p23[:, 1024:2048],
                    b1s_sb[b][:, 1:2], 0.0, ADD, MAX,
                )
                h1_g[g] = h1

                zc = zc_sb[b]
                hc = hcp.tile([D, 1024], BF16, tag="hc")
                pc = psc.tile([D, 1024], F32, tag="pc")
                for par in range(2):
                    nc.tensor.matmul(
                        pc[:, 512 * par : 512 * (par + 1)],
                        wc_sb[64 * par : 64 * par + 64, :],
                        zc[64 * par : 64 * par + 64, 512 * o : 512 * (o + 1)],
                        start=True, stop=True,
                    )
                nc.scalar.activation(hc[:], pc[:], AF.Relu, bias=bc_sb[:, 0:1])
                hc_g[g] = hc

            def s4(g):
                b, o = divmod(g, NOCT)
                h1 = h1_g.pop(g)
                hc = hc_g.pop(g)
                if g % 2 == 0:
                    p4_cur[0] = ps4.tile([D, 512], F32, name="p4",
                                         tag="p4")
                p4 = p4_cur[0]
                # the two accumulation chains (par 0/1) target different PE
                # column strips; interleaving lets them run concurrently
                def mm(par, step, start, stop):
                    s = (g % 2) * 2 + par
                    tp = (0, 32 * s)
                    if step == 1:
                        nc.tensor.matmul(
                            p4[32 * s : 32 * s + 1, :], wo_sb[:],
                            hc[:, 512 * par : 512 * (par + 1)],
                            start=start, stop=stop, tile_position=tp,
                        )
                    else:
                        hh = step // 2
                        nc.tensor.matmul(
                            p4[32 * s : 32 * s + 2, :], w2_sb[b][:, hh, :],
                            h1[:, 1024 * hh + 512 * par :
                               1024 * hh + 512 * (par + 1)],
                            start=start, stop=stop, tile_position=tp,
                        )
                for par in range(2):
                    for step in range(3):
                        mm(par, step, step == 0, step == 2)
                if g % 2 == 1:
                    sto = stop.tile([D, 512], F32, tag="sto")
                    nc.scalar.activation(sto[:], p4[:], AF.Identity,
                                         bias=bq_sb[b][:, 0:1])
                    stov = sto[:].rearrange("(s r) f -> s r f", r=32)
                    r0 = b * 16 + (o - 1) * 2
                    nc.sync.dma_start(st_mu[r0 : r0 + 4, :], stov[:, 0])
                    nc.sync.dma_start(st_sig[r0 : r0 + 4, :], stov[:, 1])

            # filler MMs bridge the PE-idle stretches while the software
            # pipeline fills, so the HAM never re-throttles
            prolog_fill = {0: 16, 1: 14, 2: 10, 3: 4}
            for g in range(NG + 3):
                if g < NG:
                    b, o = divmod(g, NOCT)
                    if o == 4 and b + 1 < NB:
                        load_b(b + 1)
                    stage1(g)
                if g in prolog_fill:
                    fillers(prolog_fill[g])
                if 2 <= g < NG + 2:
                    s23_c1(g - 2)
                if 3 <= g:
                    s4(g - 3)

            # ---- tail: softplus + outputs (b2/bo already added at evac) ----
            ex = finp.tile([NB * 16, 512], F32)
            nc.scalar.activation(ex[:], st_sig[:], AF.Exp)
            nc.scalar.activation(st_sig[:], ex[:], AF.Ln, bias=1.0)
            nc.vector.tensor_scalar_add(st_sig[:], st_sig[:], 0.01)

            # st row r = b*16 + o*2 + u covers t = o*8 + u*4 + j, cols (j, i)
            nc.sync.dma_start(
                mu_d[:].rearrange("b (qb tq) i -> (b qb) tq i", tq=4),
                st_mu[:].rearrange("p (tq i) -> p tq i", i=D),
            )
            nc.sync.dma_start(
                sg_d[:].rearrange("b (qb tq) i -> (b qb) tq i", tq=4),
                st_sig[:].rearrange("p (tq i) -> p tq i", i=D),
            )

    nc.compile()
    return nc


def _get_nc():
    if "nc" not in _CACHE:
        _CACHE["nc"] = _build()
    return _CACHE["nc"]


def _prepare_in_maps(z_signal, z_corrupt, A, regime, W_sig, b_sig, W1e, b1e,
                     W2e, b2e, Wc, bc, Wo, bo):
    z_signal = np.asarray(z_signal, dtype=np.float32)
    z_corrupt = np.asarray(z_corrupt, dtype=np.float32)
    A = np.asarray(A, dtype=np.float32)
    regime = np.asarray(regime)
    W_sig = np.asarray(W_sig, dtype=np.float32)
    b_sig = np.asarray(b_sig, dtype=np.float32)
    W1e = np.asarray(W1e, dtype=np.float32)
    b1e = np.asarray(b1e, dtype=np.float32)
    W2e = np.asarray(W2e, dtype=np.float32)
    b2e = np.asarray(b2e, dtype=np.float32)
    Wc = np.asarray(Wc, dtype=np.float32)
    bc = np.asarray(bc, dtype=np.float32)
    Wo = np.asarray(Wo, dtype=np.float32)
    bo = np.asarray(bo, dtype=np.float32)

    eidx = np.where(regime >= NE, 0, regime).astype(np.int32)

    # ---- host weight transforms (env tables, replicated to all cores) ----
    ai = _bf16(A)                                                  # [D, D]
    w1s_half = _bf16(np.einsum("lh,ehk->elk", W_sig, W1e))         # [E, L, H]
    w1s = np.concatenate([w1s_half, w1s_half], axis=1)             # [E, D, H]
    b1s_full = np.einsum("h,ehk->ek", b_sig, W1e) + b1e            # [E, H]
    b1s = np.ascontiguousarray(
        b1s_full.reshape(NE, 2, D).transpose(0, 2, 1))             # [E, D, 2]
    w2p = _bf16(np.ascontiguousarray(
        W2e.reshape(NE, 2, D, 2).transpose(0, 2, 1, 3)))           # [E,D,2,2]
    bq = np.zeros((NE, D, 1), dtype=np.float32)
    for s in range(4):
        bq[:, 32 * s, 0] = b2e[:, 0] + bo[0]
        bq[:, 32 * s + 1, 0] = b2e[:, 1]
    wc_r = _bf16(np.concatenate([Wc] * 2, axis=0))                 # [D, H2]
    wo_r = _bf16(Wo)                                               # [H2, 1]
    bc_r = np.ascontiguousarray(bc[:, None])                       # [H2, 1]

    in_maps = []
    for c in range(N_CORES):
        b0 = c * NB
        zs = z_signal[b0 : b0 + NB]
        zc = z_corrupt[b0 : b0 + NB]
        # signal pairs are (t, t+4) within an oct: pair pr = o*4+j holds
        # t = o*8 + u*4 + j at col pr*128 + u*64 + l
        zzt_ = zs.transpose(0, 2, 1, 3)                  # b,i,t,l
        zzt_ = zzt_.reshape(NB, D, NOCT, 2, 4, L)        # b,i,o,u,j,l
        zzi = _bf16(np.ascontiguousarray(
            zzt_.transpose(0, 1, 2, 4, 3, 5)).reshape(NB, D, T * L))
        # corrupt: [nb, T, D, L] -> [nb, (u,l), (o,j,i)], t = o*8 + u*4 + j
        zct = zc.reshape(NB, NOCT, 2, 4, D, L)           # b,o,u,j,i,l
        zct = zct.transpose(0, 2, 5, 1, 3, 4)            # b,u,l,o,j,i
        zci = _bf16(np.ascontiguousarray(zct).reshape(NB, D, NOCT * 512))
        in_maps.append({
            "zzi": zzi,
            "zci": zci,
            "ai": ai,
            "reg": eidx[None, b0 : b0 + NB],
            "w1s": w1s,
            "b1s": b1s,
            "w2p": w2p,
            "bq": bq,
            "wc": wc_r,
            "bc": bc_r,
            "wo": wo_r,
        })
    return in_maps


def kernel(z_signal, z_corrupt, A, regime, W_sig, b_sig, W1e, b1e, W2e, b2e,
           Wc, bc, Wo, bo):
    from concourse.bass_utils import run_bass_kernel_spmd

    in_maps = _prepare_in_maps(z_signal, z_corrupt, A, regime, W_sig, b_sig,
                               W1e, b1e, W2e, b2e, Wc, bc, Wo, bo)
    nc = _get_nc()
    res = run_bass_kernel_spmd(nc, in_maps, core_ids=list(range(N_CORES)))

    mu = np.concatenate([r["mu"] for r in res.results], axis=0)
    sigma = np.concatenate([r["sg"] for r in res.results], axis=0)
    return mu, sigma


def run_traced(inputs_np):
    from concourse.bass_utils import run_bass_kernel_spmd

    in_maps = _prepare_in_maps(**inputs_np)
    nc = _get_nc()
    return run_bass_kernel_spmd(
        nc, in_maps, core_ids=list(range(N_CORES)), trace=True
    )
